# revision 42
# baseline (speedup 1.0000x reference)
"""Trainium2 Bass kernel for nn_MultiHeadAttention_824633721543.

MHA with periodic prefix mask: allowed iff (q % 256) >= (k % 256).
B=2, S=2048, D=768, H=12, Dk=64, WINDOW=256.

Sharding: 8 cores = 2 batches x 4 head-groups (3 heads each). Each core
computes q/k/v projections for its heads, the masked softmax attention, and
a partial O-projection; the host sums the 4 partials per batch and adds bo.

v2 design (all activations/weights bf16, PSUM accumulation fp32):
  - scores computed as S^T [k,q]; q columns tile-permuted (even 128-tiles |
    odd 128-tiles) so masks reduce to one shared 128x128 triu tile.
  - h0/h1 q,k stacked on partition halves of one [128,S] tile; score matmuls
    for both heads issued as K=64 row-tiled pairs (tile_position (0,0)/(64,0))
    that run concurrently in the PE array.
  - h2 q,k duplicated onto both partition halves (free: the projection
    matmul's stationary has spare M), so h2's two q-groups pack the same way.
  - exp on ACT (scale=1/8), bf16 out; mask = DVE bf16 multiply (4x mode).
  - P@V with [V|1] stationary -> out^T plus denominator row, accumulated over
    windows in PSUM; normalization via K=1 broadcast matmul + DVE.
  - stage A for h2 is emitted AFTER stage B of h0/h1 so the Tile scheduler
    uses it as PE filler while ACT chews exp (keeps the PE HAM-warm).
  - O-projection: h0/h1 as one K=128 matmul, h2 K=64; output DMA'd as bf16.
"""

import sys

sys.path.insert(0, "/opt/trn_rl_repo")

import numpy as np
import ml_dtypes

BF16 = ml_dtypes.bfloat16

B = 2
S = 2048
D = 768
DK = 64
WIN = 256
NW = S // WIN   # 8 windows
NHC = 3         # heads per core
DH = NHC * DK   # 192
NT = S // 128   # 16 q tiles

_CACHE = {}


def _build_program():
    import concourse.tile as tile
    from concourse import mybir, bacc
    from contextlib import ExitStack

    f32 = mybir.dt.float32
    f32r = mybir.dt.float32r
    bf16 = mybir.dt.bfloat16
    Exp = mybir.ActivationFunctionType.Exp
    Ident = mybir.ActivationFunctionType.Identity
    mult = mybir.AluOpType.mult

    nc = bacc.Bacc("TRN2", target_bir_lowering=False, debug=False)

    # xB: [128, 12288] packed as (n 4, k 6, s 512); wQ: [128, (k 6, 256)] q/k
    # weights for h0/h1; wR: [128, (k 6, [w2 256 | wv 192])]
    xB = nc.dram_tensor("xB", [128, 12288], bf16, kind="ExternalInput").ap()
    wQ = nc.dram_tensor("wQ", [128, 1536], bf16, kind="ExternalInput").ap()
    wR = nc.dram_tensor("wR", [128, 2688], bf16, kind="ExternalInput").ap()
    wo1 = nc.dram_tensor("wo1", [128, D], bf16, kind="ExternalInput").ap()
    wo2 = nc.dram_tensor("wo2", [128, D], bf16, kind="ExternalInput").ap()
    btA = nc.dram_tensor("btA", [128, 1], f32, kind="ExternalInput").ap()
    btB = nc.dram_tensor("btB", [128, 1], f32, kind="ExternalInput").ap()
    btC = nc.dram_tensor("btC", [128, 1], f32, kind="ExternalInput").ap()
    btD = nc.dram_tensor("btD", [128, 1], f32, kind="ExternalInput").ap()
    bvb = nc.dram_tensor("bvb", [128, 192], f32, kind="ExternalInput").ap()
    triu = nc.dram_tensor("triu", [128, 128], bf16, kind="ExternalInput").ap()
    onesb = nc.dram_tensor("onesb", [128, 64], bf16, kind="ExternalInput").ap()
    onesr = nc.dram_tensor("onesr", [1, 64], f32r, kind="ExternalInput").ap()
    out = nc.dram_tensor("out", [S, D], bf16, kind="ExternalOutput").ap()

    with tile.TileContext(nc) as tc, ExitStack() as ctx:
        consts = ctx.enter_context(tc.tile_pool(name="consts", bufs=1))
        qkv = ctx.enter_context(tc.tile_pool(name="qkv", bufs=1))
        xtp = ctx.enter_context(tc.tile_pool(name="xtp", bufs=1))

        xB_sb = xtp.tile([128, 12288], bf16, tag="xB")
        wQ_sb = consts.tile([128, 1536], bf16, tag="wQ")
        wR_sb = consts.tile([128, 2688], bf16, tag="wR")
        # DMA order: wQ, xB n0/n1 (stage-A critical path), wR, xB n2/n3.
        # Odd chunks ride the Activation HWDGE queue in parallel with sync's.
        nc.sync.dma_start(out=wQ_sb, in_=wQ)
        nc.sync.dma_start(out=xB_sb[:, 0:3072], in_=xB[:, 0:3072])
        nc.scalar.dma_start(out=xB_sb[:, 3072:6144], in_=xB[:, 3072:6144])
        nc.sync.dma_start(out=wR_sb, in_=wR)
        nc.sync.dma_start(out=xB_sb[:, 6144:9216], in_=xB[:, 6144:9216])
        nc.scalar.dma_start(out=xB_sb[:, 9216:12288], in_=xB[:, 9216:12288])

        def xn_sl(k, n):
            return xB_sb[:, 3072 * n + 512 * k:3072 * n + 512 * k + 512]

        def xst_sl(k, st):
            c = 3072 * (st // 4) + 512 * k + 128 * (st % 4)
            return xB_sb[:, c:c + 128]

        w1_sb = [wQ_sb[:, 256 * k:256 * k + 256] for k in range(6)]
        w2_sb = [wR_sb[:, 448 * k:448 * k + 256] for k in range(6)]
        wv_sb = [wR_sb[:, 448 * k + 256:448 * k + 448] for k in range(6)]
        wo1_sb = consts.tile([128, D], bf16, tag="wo1")
        wo2_sb = consts.tile([128, D], bf16, tag="wo2")
        nc.sync.dma_start(out=wo1_sb, in_=wo1)
        nc.sync.dma_start(out=wo2_sb, in_=wo2)
        btA_sb = consts.tile([128, 1], f32, tag="btA")
        btB_sb = consts.tile([128, 1], f32, tag="btB")
        btC_sb = consts.tile([128, 1], f32, tag="btC")
        btD_sb = consts.tile([128, 1], f32, tag="btD")
        nc.sync.dma_start(out=btA_sb, in_=btA)
        nc.sync.dma_start(out=btB_sb, in_=btB)
        nc.sync.dma_start(out=btC_sb, in_=btC)
        nc.sync.dma_start(out=btD_sb, in_=btD)
        bvb_sb = consts.tile([128, 192], f32, tag="bvb")
        nc.sync.dma_start(out=bvb_sb, in_=bvb)
        triu_sb = consts.tile([128, 128], bf16, tag="triu")
        nc.sync.dma_start(out=triu_sb, in_=triu)
        onesb_sb = consts.tile([128, 64], bf16, tag="onesb")
        nc.sync.dma_start(out=onesb_sb, in_=onesb)
        onesr_sb = consts.tile([1, 64], f32r, tag="onesr")
        nc.sync.dma_start(out=onesr_sb, in_=onesr)

        # ---- long-lived activation tiles (bf16) ----
        qAB = qkv.tile([128, S], bf16, tag="qAB")  # [qT_h0|qT_h1], q-permuted
        kAB = qkv.tile([128, S], bf16, tag="kAB")  # [kT_h0|kT_h1], natural
        qC2 = qkv.tile([128, S], bf16, tag="qC2")  # qT_h2 dup'd, permuted
        kC2 = qkv.tile([128, S], bf16, tag="kC2")  # kT_h2 dup'd, natural
        v_sb = [qkv.tile([128, 195], bf16, tag=f"v{i}", name=f"vsb{i}")
                for i in range(NT)]
        attn01 = qkv.tile([128, S], bf16, tag="attn01")  # h0 0-63, h1 64-127
        attn2 = qkv.tile([128, S], bf16, tag="attn2")  # h2 dup'd on halves

        def permuted_copy(dst, ps, n, bias, eng="dve"):
            """psum 512-span n -> dst cols with even/odd tile permutation.
            eng="act" runs the biased copy on the scalar engine (Identity
            activation with per-partition bias AP) to offload DVE."""
            pr3 = ps.rearrange("p (c two k) -> p c two k", two=2, k=128)
            d0 = dst[:, 256 * n:256 * n + 256].rearrange(
                "p (c k) -> p c k", k=128)
            d1 = dst[:, 1024 + 256 * n:1024 + 256 * n + 256].rearrange(
                "p (c k) -> p c k", k=128)
            if eng == "act":
                nc.scalar.activation(out=d0, in_=pr3[:, :, 0, :], func=Ident,
                                     bias=bias)
                nc.scalar.activation(out=d1, in_=pr3[:, :, 1, :], func=Ident,
                                     bias=bias)
            else:
                nc.vector.tensor_scalar_add(out=d0, in0=pr3[:, :, 0, :],
                                            scalar1=bias)
                nc.vector.tensor_scalar_add(out=d1, in0=pr3[:, :, 1, :],
                                            scalar1=bias)

        triu_b = triu_sb.unsqueeze(1)

        with tc.tile_pool(name="sc", bufs=2, space="PSUM") as scp, \
             tc.tile_pool(name="po", bufs=2, space="PSUM") as pop, \
             tc.tile_pool(name="aps", bufs=2, space="PSUM") as aps, \
             tc.tile_pool(name="pt", bufs=12) as ptp, \
             tc.tile_pool(name="ost", bufs=3) as ost, \
             tc.tile_pool(name="nrm", bufs=6) as nrm:

            def a01_chunk(n):
                xn = [xn_sl(k, n) for k in range(6)]
                eng = "act" if n < 2 else "dve"
                psa = aps.tile([128, 512], f32, tag="apsA")
                for k in range(6):
                    nc.tensor.matmul(psa, w1_sb[k][:, 0:128], xn[k],
                                     start=(k == 0), stop=(k == 5))
                permuted_copy(qAB, psa, n, btA_sb, eng=eng)
                psb = aps.tile([128, 512], f32, tag="apsA")
                for k in range(6):
                    nc.tensor.matmul(psb, w1_sb[k][:, 128:256], xn[k],
                                     start=(k == 0), stop=(k == 5))
                if n < 2:
                    nc.scalar.activation(out=kAB[:, 512 * n:512 * (n + 1)],
                                         in_=psb, func=Ident, bias=btB_sb)
                else:
                    nc.vector.tensor_scalar_add(
                        out=kAB[:, 512 * n:512 * (n + 1)], in0=psb,
                        scalar1=btB_sb)

            def a2_chunk(j):
                """j 0..3: qC2 chunk j; j 4..7: kC2 chunk j-4."""
                n = j % 4
                xn = [xn_sl(k, n) for k in range(6)]
                ps = aps.tile([128, 512], f32, tag="apsA")
                cols = slice(0, 128) if j < 4 else slice(128, 256)
                for k in range(6):
                    nc.tensor.matmul(ps, w2_sb[k][:, cols], xn[k],
                                     start=(k == 0), stop=(k == 5))
                if j < 4:
                    permuted_copy(qC2, ps, n, btC_sb)
                else:
                    nc.vector.tensor_scalar_add(
                        out=kC2[:, 512 * n:512 * (n + 1)], in0=ps,
                        scalar1=btD_sb)

            def v_proj(st):
                psv = aps.tile([128, 192], f32, tag="apsA")
                for k in range(6):
                    nc.tensor.matmul(psv, xst_sl(k, st), wv_sb[k],
                                     start=(k == 0), stop=(k == 5))
                vt = v_sb[st]
                nc.vector.tensor_tensor(
                    out=vt.rearrange("p (h c) -> p h c", c=65)[:, :, 0:64],
                    in0=psv.rearrange("p (h c) -> p h c", c=64),
                    in1=bvb_sb.rearrange("p (h c) -> p h c", c=64),
                    op=mybir.AluOpType.add)

            def norm(po, dst_rows, dst_cols, den_eng=None):
                """po [65,512]: rows 0-63 = out^T, row 64 = denom."""
                den = nrm.tile([1, 512], f32r, tag="den")
                if den_eng == "act":
                    nc.scalar.copy(out=den, in_=po[64:65, :])
                else:
                    nc.vector.tensor_copy(out=den, in_=po[64:65, :])
                dps = aps.tile([64, 512], f32, tag="apsA")
                nc.tensor.matmul(dps, onesr_sb, den, start=True, stop=True)
                rec = nrm.tile([64, 512], f32, tag="rec")
                nc.vector.reciprocal_approx_fast(out=rec, in_=dps)
                nc.vector.tensor_tensor(
                    out=dst_rows[:, dst_cols], in0=po[0:64, :], in1=rec,
                    op=mult)

            def exp_unit(sc, mask_lo, mask_hi):
                pt = ptp.tile([128, 1024], bf16, tag="pt")
                nc.scalar.activation(out=pt, in_=sc, func=Exp, scale=0.125)
                if mask_lo and mask_hi:
                    p3 = pt.rearrange("p (c k) -> p c k", k=128)
                    nc.vector.tensor_mul(
                        out=p3, in0=p3,
                        in1=triu_b.broadcast_to([128, 8, 128]))
                elif mask_lo or mask_hi:
                    off = 0 if mask_lo else 512
                    p3 = pt[:, off:off + 512].rearrange(
                        "p (c k) -> p c k", k=128)
                    nc.vector.tensor_mul(
                        out=p3, in0=p3,
                        in1=triu_b.broadcast_to([128, 4, 128]))
                return pt

            # ---- B01: heads h0,h1 on partition strips, PV delayed 1 unit ----
            def b01(g, hh, pre=None, den_eng=None):
                qc = slice(1024 * g + 512 * hh, 1024 * g + 512 * hh + 512)
                po0 = pop.tile([65, 512], f32, tag="po")
                po1 = pop.tile([65, 512], f32, tag="po")
                units = []  # (kslice, v_idx, masked)
                for w in range(NW):
                    units.append((slice(WIN * w, WIN * w + 128), 2 * w,
                                  g == 0))
                    if g == 1:
                        units.append((slice(WIN * w + 128, WIN * w + 256),
                                      2 * w + 1, True))
                nu = len(units)

                def pv(i, pt):
                    vsl = v_sb[units[i][1]]
                    nc.tensor.matmul(po0, vsl[:, 0:65], pt[:, 0:512],
                                     start=(i == 0), stop=(i == nu - 1))
                    nc.tensor.matmul(po1, vsl[:, 65:130], pt[:, 512:1024],
                                     start=(i == 0), stop=(i == nu - 1))

                prev = None
                for i, (ksl, vidx, msk) in enumerate(units):
                    sc = scp.tile([128, 1024], f32, tag="sc")
                    nc.tensor.matmul(sc[:, 0:512], kAB[0:64, ksl],
                                     qAB[0:64, qc], start=True, stop=True)
                    nc.tensor.matmul(sc[:, 512:1024], kAB[64:128, ksl],
                                     qAB[64:128, qc], start=True, stop=True)
                    pt = exp_unit(sc, mask_lo=msk, mask_hi=msk)
                    if prev is not None:
                        pv(*prev)
                    if pre is not None:
                        pre(i)
                    prev = (i, pt)
                pv(*prev)
                cols = slice(1024 * g + 512 * hh, 1024 * g + 512 * hh + 512)
                norm(po0, attn01[0:64, :], cols, den_eng)
                norm(po1, attn01[64:128, :], cols, den_eng)

            # ---- B2: head h2 quarter-pass; two half-blocks per sc tile ----
            def b2q(g, hh, pre=None, den_eng=None):
                qc = slice(1024 * g + 512 * hh, 1024 * g + 512 * hh + 512)
                pg = pop.tile([65, 512], f32, tag="po")
                halves = []  # (kslice, v_idx, masked)
                for w in range(NW):
                    if g == 0:
                        halves.append((slice(WIN * w, WIN * w + 128), 2 * w,
                                       True))
                    else:
                        halves.append((slice(WIN * w, WIN * w + 128), 2 * w,
                                       False))
                        halves.append((slice(WIN * w + 128, WIN * w + 256),
                                       2 * w + 1, True))
                nh = len(halves)

                def pvs(j, pt):
                    for s in (0, 1):
                        i = j + s
                        nc.tensor.matmul(
                            pg, v_sb[halves[i][1]][:, 130:195],
                            pt[:, 512 * s:512 * (s + 1)],
                            start=(i == 0), stop=(i == nh - 1))

                prev = None
                for j in range(0, nh, 2):
                    pair = halves[j:j + 2]
                    sc = scp.tile([128, 1024], f32, tag="sc")
                    for s, (ksl, _, _) in enumerate(pair):
                        nc.tensor.matmul(
                            sc[:, 512 * s:512 * (s + 1)],
                            kC2[64 * s:64 * (s + 1), ksl],
                            qC2[64 * s:64 * (s + 1), qc],
                            start=True, stop=True)
                    pt = exp_unit(sc, mask_lo=pair[0][2], mask_hi=pair[1][2])
                    if prev is not None:
                        pvs(*prev)
                    if pre is not None:
                        pre(j // 2)
                    prev = (j, pt)
                pvs(*prev)
                norm(pg, attn2[0:64, :], qc, den_eng)
                nc.vector.tensor_copy(out=attn2[64:128, qc],
                                      in_=attn2[0:64, qc])

            # ---- C: O-projection for one permuted 128-col tile ----
            def c_tile(p):
                pso = scp.tile([128, D], f32, tag="sc")
                pcols = slice(128 * p, 128 * (p + 1))
                half = 64 * (p % 2)  # alternate attn2/wo2 strip per p parity
                for (n0, n1) in ((0, 512), (512, 768)):
                    nc.tensor.matmul(pso[:, n0:n1], attn01[:, pcols],
                                     wo1_sb[:, n0:n1], start=True, stop=False)
                    nc.tensor.matmul(pso[:, n0:n1],
                                     attn2[half:half + 64, pcols],
                                     wo2_sb[half:half + 64, n0:n1],
                                     start=False, stop=True)
                ot = ost.tile([128, D], bf16, tag="ot")
                if p % 2 == 0 or p >= 12:
                    nc.scalar.copy(out=ot, in_=pso)
                else:
                    nc.vector.tensor_copy(out=ot, in_=pso)
                t = 2 * p if p < 8 else 2 * (p - 8) + 1
                nc.sync.dma_start(out=out[128 * t:128 * (t + 1), :], in_=ot)

            # ---- emission schedule ----
            # PE warmup: dummy matmuls on a memset tile, no DMA dependency —
            # keeps the PE HAM activity monitor busy through the DMA lead-in
            # so stage A runs at 2.4 GHz instead of the cold 1.2 GHz.
            wu_in = ost.tile([128, 128], bf16, tag="wu")
            nc.vector.memset(wu_in, 0.0)
            wu_ps = scp.tile([128, 128], f32, tag="sc")
            for _ in range(60):
                nc.tensor.matmul(wu_ps, wu_in, wu_in, start=True, stop=True)
            for st in range(NT):
                nc.vector.tensor_copy(
                    out=v_sb[st].rearrange(
                        "p (h c) -> p h c", c=65)[:, :, 64:65],
                    in_=onesb_sb[:, 0:3].unsqueeze(2))

            a01_chunk(0)
            a01_chunk(1)

            def pre00(i):
                # even-indexed V tiles (the only ones grp0 needs) + rest of A01
                if i == 1:
                    a01_chunk(2)
                if i == 2:
                    a01_chunk(3)
                if i < NW:
                    v_proj(2 * i)

            def pre01(i):
                # odd V tiles + h2 projections, filler for the second g0 pass
                if i < NW:
                    v_proj(2 * i + 1)
                    a2_chunk(i)

            def pre10(i):
                if i % 2 == 0 and i // 2 < 8:
                    c_tile(i // 2)

            def pre21(i):
                if i < 4:
                    c_tile(8 + i)

            b01(0, 0, pre=pre00, den_eng="act")
            b01(0, 1, pre=pre01, den_eng="act")
            b2q(0, 0, den_eng="act")
            b2q(0, 1, den_eng="act")
            b01(1, 0, pre=pre10)
            b01(1, 1)
            b2q(1, 0)
            b2q(1, 1, pre=pre21)
            for p in range(12, 16):
                c_tile(p)

    nc.compile()
    return nc


def _prep_core_inputs(inputs, c):
    x = inputs["x"]
    Wq, bq = inputs["Wq"], inputs["bq"]
    Wk, bk = inputs["Wk"], inputs["bk"]
    Wv, bv = inputs["Wv"], inputs["bv"]
    Wo = inputs["Wo"]
    b = c // 4
    r0 = (c % 4) * DH  # first feature row of this core's 192-row head block

    xT = np.asarray(x[b]).T.astype(np.float32)  # [768, 2048]
    W1 = np.concatenate(
        [Wq[r0:r0 + 128].T, Wk[r0:r0 + 128].T], axis=1)
    q2 = Wq[r0 + 128:r0 + 192].T
    k2 = Wk[r0 + 128:r0 + 192].T
    W2 = np.concatenate([q2, q2, k2, k2], axis=1)
    Wvp = Wv[r0:r0 + 192].T
    # packed x: [128, (n 4, k 6, s 512)]
    xBp = np.zeros((128, 12288), np.float32)
    for n in range(4):
        for k in range(6):
            xBp[:, 3072 * n + 512 * k:3072 * n + 512 * (k + 1)] = \
                xT[128 * k:128 * (k + 1), 512 * n:512 * (n + 1)]
    # packed weights: wQ [128, (k 6, w1 256)]; wR [128, (k 6, [w2 256|wv 192])]
    wQp = np.zeros((128, 1536), np.float32)
    wRp = np.zeros((128, 2688), np.float32)
    for k in range(6):
        wQp[:, 256 * k:256 * (k + 1)] = W1[128 * k:128 * (k + 1)]
        wRp[:, 448 * k:448 * k + 256] = W2[128 * k:128 * (k + 1)]
        wRp[:, 448 * k + 256:448 * k + 448] = Wvp[128 * k:128 * (k + 1)]
    bq2 = bq[r0 + 128:r0 + 192]
    bk2 = bk[r0 + 128:r0 + 192]
    bvb = np.zeros((128, 192), np.float32)
    for h in range(NHC):
        bvb[:, 64 * h:64 * h + 64] = bv[r0 + 64 * h:r0 + 64 * (h + 1)][None, :]

    return dict(
        xB=xBp.astype(BF16), wQ=wQp.astype(BF16), wR=wRp.astype(BF16),
        wo1=np.ascontiguousarray(Wo[:, r0:r0 + 128].T).astype(BF16),
        wo2=np.ascontiguousarray(np.concatenate(
            [Wo[:, r0 + 128:r0 + 192].T] * 2, axis=0)).astype(BF16),
        btA=np.ascontiguousarray(bq[r0:r0 + 128].reshape(128, 1)).astype(np.float32),
        btB=np.ascontiguousarray(bk[r0:r0 + 128].reshape(128, 1)).astype(np.float32),
        btC=np.concatenate([bq2, bq2]).reshape(128, 1).astype(np.float32),
        btD=np.concatenate([bk2, bk2]).reshape(128, 1).astype(np.float32),
        bvb=np.ascontiguousarray(bvb),
        triu=np.triu(np.ones((128, 128), np.float32)).astype(BF16),
        onesb=np.ones((128, 64), np.float32).astype(BF16),
        onesr=np.ones((1, 64), np.float32),
    )


def _install_ntff_hook():
    """Register antenv.axon_hooks with a ctypes NTFF profile hook so
    run_bass_kernel_spmd(trace=True) can capture device-side exec time."""
    import types, ctypes, contextlib

    try:
        import antenv.axon_hooks  # noqa: F401
        return
    except ImportError:
        pass
    so_path = "/opt/axon/libaxon_pjrt.so"
    lib = ctypes.CDLL(so_path)
    if not hasattr(lib, "axon_start_nrt_profile"):
        return
    lib.axon_start_nrt_profile.argtypes = [
        ctypes.POINTER(ctypes.c_int64), ctypes.c_size_t]
    lib.axon_start_nrt_profile.restype = ctypes.c_int64
    lib.axon_stop_nrt_profile.argtypes = [ctypes.c_char_p]
    lib.axon_stop_nrt_profile.restype = ctypes.c_int64

    @contextlib.contextmanager
    def _hook(output_dir, device_ids):
        import jax
        jax.devices()
        if device_ids:
            ids = (ctypes.c_int64 * len(device_ids))(*device_ids)
            rc = lib.axon_start_nrt_profile(ids, len(device_ids))
        else:
            rc = lib.axon_start_nrt_profile(None, 0)
        if rc != 0:
            raise RuntimeError(f"axon_start_nrt_profile rc={rc}")
        try:
            yield
        finally:
            n = lib.axon_stop_nrt_profile(str(output_dir).encode())
            print(f"profile: {n} file(s) written to {output_dir}")

    mod = types.ModuleType("antenv.axon_hooks")
    mod.get_axon_ntff_profile_hook = lambda: _hook
    mod.set_axon_ntff_profile_hook = lambda h: None
    sys.modules["antenv.axon_hooks"] = mod
    import antenv
    antenv.axon_hooks = mod


def kernel(**inputs):
    import os
    from concourse import bass_utils

    if "nc" not in _CACHE:
        _CACHE["nc"] = _build_program()
    nc = _CACHE["nc"]

    trace = bool(os.environ.get("MHA_TRACE"))
    kwargs = {}
    if trace:
        _install_ntff_hook()
        kwargs = dict(trace=True, tmpdir="/tmp/mha_trace")
        os.makedirs("/tmp/mha_trace", exist_ok=True)

    in_maps = [_prep_core_inputs(inputs, c) for c in range(8)]
    res = bass_utils.run_bass_kernel_spmd(
        nc, in_maps, core_ids=list(range(8)), **kwargs)
    _CACHE["last_results"] = res
    if trace and res.exec_time_ns is not None:
        print(f"HW exec time: {res.exec_time_ns} ns")
    out = np.zeros((B, S, D), np.float32)
    for c in range(8):
        out[c // 4] += res.results[c]["out"].astype(np.float32)
    out += np.asarray(inputs["bo"], np.float32).reshape(1, 1, D)
    return out


# revision 43
# speedup vs baseline: 1.2132x; 1.2132x over previous
"""Trainium2 Bass kernel for nn_MultiHeadAttention_824633721543.

MHA with periodic prefix mask: allowed iff (q % 256) >= (k % 256).
B=2, S=2048, D=768, H=12, Dk=64, WINDOW=256.

Sharding: 8 cores = 2 batches x 4 head-groups (3 heads each). Each core
computes q/k/v projections for its heads, the masked softmax attention, and
a partial O-projection; the host sums the 4 partials per batch and adds bo.

v2 design (all activations/weights bf16, PSUM accumulation fp32):
  - scores computed as S^T [k,q]; q columns tile-permuted (even 128-tiles |
    odd 128-tiles) so masks reduce to one shared 128x128 triu tile.
  - h0/h1 q,k stacked on partition halves of one [128,S] tile; score matmuls
    for both heads issued as K=64 row-tiled pairs (tile_position (0,0)/(64,0))
    that run concurrently in the PE array.
  - h2 q,k duplicated onto both partition halves (free: the projection
    matmul's stationary has spare M), so h2's two q-groups pack the same way.
  - exp on ACT (scale=1/8), bf16 out; mask = DVE bf16 multiply (4x mode).
  - P@V with [V|1] stationary -> out^T plus denominator row, accumulated over
    windows in PSUM; normalization via K=1 broadcast matmul + DVE.
  - stage A for h2 is emitted AFTER stage B of h0/h1 so the Tile scheduler
    uses it as PE filler while ACT chews exp (keeps the PE HAM-warm).
  - O-projection: h0/h1 as one K=128 matmul, h2 K=64; output DMA'd as bf16.
"""

import sys

sys.path.insert(0, "/opt/trn_rl_repo")

import numpy as np
import ml_dtypes

BF16 = ml_dtypes.bfloat16

B = 2
S = 2048
D = 768
DK = 64
WIN = 256
NW = S // WIN   # 8 windows
NHC = 3         # heads per core
DH = NHC * DK   # 192
NT = S // 128   # 16 q tiles

_CACHE = {}


def _build_program():
    import concourse.tile as tile
    from concourse import mybir, bacc
    from contextlib import ExitStack

    f32 = mybir.dt.float32
    f32r = mybir.dt.float32r
    bf16 = mybir.dt.bfloat16
    Exp = mybir.ActivationFunctionType.Exp
    Ident = mybir.ActivationFunctionType.Identity
    mult = mybir.AluOpType.mult

    nc = bacc.Bacc("TRN2", target_bir_lowering=False, debug=False)

    # xB: [128, 12288] packed as (n 4, k 6, s 512); wQ: [128, (k 6, 256)] q/k
    # weights for h0/h1; wR: [128, (k 6, [w2 256 | wv 192])]
    xB = nc.dram_tensor("xB", [128, 12288], bf16, kind="ExternalInput").ap()
    wQ = nc.dram_tensor("wQ", [128, 1536], bf16, kind="ExternalInput").ap()
    wR = nc.dram_tensor("wR", [128, 2688], bf16, kind="ExternalInput").ap()
    wo1 = nc.dram_tensor("wo1", [128, D], bf16, kind="ExternalInput").ap()
    wo2 = nc.dram_tensor("wo2", [128, D], bf16, kind="ExternalInput").ap()
    btA = nc.dram_tensor("btA", [128, 1], f32, kind="ExternalInput").ap()
    btB = nc.dram_tensor("btB", [128, 1], f32, kind="ExternalInput").ap()
    btC = nc.dram_tensor("btC", [128, 1], f32, kind="ExternalInput").ap()
    btD = nc.dram_tensor("btD", [128, 1], f32, kind="ExternalInput").ap()
    bvb = nc.dram_tensor("bvb", [128, 192], f32, kind="ExternalInput").ap()
    triu = nc.dram_tensor("triu", [128, 128], bf16, kind="ExternalInput").ap()
    onesb = nc.dram_tensor("onesb", [128, 64], bf16, kind="ExternalInput").ap()
    onesr = nc.dram_tensor("onesr", [1, 64], f32r, kind="ExternalInput").ap()
    out = nc.dram_tensor("out", [S, D], bf16, kind="ExternalOutput").ap()

    with tile.TileContext(nc) as tc, ExitStack() as ctx:
        consts = ctx.enter_context(tc.tile_pool(name="consts", bufs=1))
        qkv = ctx.enter_context(tc.tile_pool(name="qkv", bufs=1))
        xtp = ctx.enter_context(tc.tile_pool(name="xtp", bufs=1))

        xB_sb = xtp.tile([128, 12288], bf16, tag="xB")
        wQ_sb = consts.tile([128, 1536], bf16, tag="wQ")
        wR_sb = consts.tile([128, 2688], bf16, tag="wR")
        # DMA order: wQ, xB n0/n1 (stage-A critical path), wR, xB n2/n3.
        # Odd chunks ride the Activation HWDGE queue in parallel with sync's.
        nc.sync.dma_start(out=wQ_sb, in_=wQ)
        nc.sync.dma_start(out=xB_sb[:, 0:3072], in_=xB[:, 0:3072])
        nc.scalar.dma_start(out=xB_sb[:, 3072:6144], in_=xB[:, 3072:6144])
        nc.sync.dma_start(out=wR_sb, in_=wR)
        nc.sync.dma_start(out=xB_sb[:, 6144:9216], in_=xB[:, 6144:9216])
        nc.scalar.dma_start(out=xB_sb[:, 9216:12288], in_=xB[:, 9216:12288])

        def xn_sl(k, n):
            return xB_sb[:, 3072 * n + 512 * k:3072 * n + 512 * k + 512]

        def xst_sl(k, st):
            c = 3072 * (st // 4) + 512 * k + 128 * (st % 4)
            return xB_sb[:, c:c + 128]

        w1_sb = [wQ_sb[:, 256 * k:256 * k + 256] for k in range(6)]
        w2_sb = [wR_sb[:, 448 * k:448 * k + 256] for k in range(6)]
        wv_sb = [wR_sb[:, 448 * k + 256:448 * k + 448] for k in range(6)]
        wo1_sb = consts.tile([128, D], bf16, tag="wo1")
        wo2_sb = consts.tile([128, D], bf16, tag="wo2")
        nc.sync.dma_start(out=wo1_sb, in_=wo1)
        nc.sync.dma_start(out=wo2_sb, in_=wo2)
        btA_sb = consts.tile([128, 1], f32, tag="btA")
        btB_sb = consts.tile([128, 1], f32, tag="btB")
        btC_sb = consts.tile([128, 1], f32, tag="btC")
        btD_sb = consts.tile([128, 1], f32, tag="btD")
        nc.sync.dma_start(out=btA_sb, in_=btA)
        nc.sync.dma_start(out=btB_sb, in_=btB)
        nc.sync.dma_start(out=btC_sb, in_=btC)
        nc.sync.dma_start(out=btD_sb, in_=btD)
        bvb_sb = consts.tile([128, 192], f32, tag="bvb")
        nc.sync.dma_start(out=bvb_sb, in_=bvb)
        triu_sb = consts.tile([128, 128], bf16, tag="triu")
        nc.sync.dma_start(out=triu_sb, in_=triu)
        onesb_sb = consts.tile([128, 64], bf16, tag="onesb")
        nc.sync.dma_start(out=onesb_sb, in_=onesb)
        onesr_sb = consts.tile([1, 64], f32r, tag="onesr")
        nc.sync.dma_start(out=onesr_sb, in_=onesr)

        # ---- long-lived activation tiles (bf16) ----
        qAB = qkv.tile([128, S], bf16, tag="qAB")  # [qT_h0|qT_h1], q-permuted
        kAB = qkv.tile([128, S], bf16, tag="kAB")  # [kT_h0|kT_h1], natural
        qC2 = qkv.tile([128, S], bf16, tag="qC2")  # qT_h2 dup'd, permuted
        kC2 = qkv.tile([128, S], bf16, tag="kC2")  # kT_h2 dup'd, natural
        v_sb = [qkv.tile([128, 195], bf16, tag=f"v{i}", name=f"vsb{i}")
                for i in range(NT)]
        attn01 = qkv.tile([128, S], bf16, tag="attn01")  # h0 0-63, h1 64-127
        attn2 = qkv.tile([128, S], bf16, tag="attn2")  # h2 dup'd on halves

        def permuted_copy(dst, ps, n, bias, eng="dve"):
            """psum 512-span n -> dst cols with even/odd tile permutation.
            eng="act" runs the biased copy on the scalar engine (Identity
            activation with per-partition bias AP) to offload DVE."""
            pr3 = ps.rearrange("p (c two k) -> p c two k", two=2, k=128)
            d0 = dst[:, 256 * n:256 * n + 256].rearrange(
                "p (c k) -> p c k", k=128)
            d1 = dst[:, 1024 + 256 * n:1024 + 256 * n + 256].rearrange(
                "p (c k) -> p c k", k=128)
            if eng == "act":
                nc.scalar.activation(out=d0, in_=pr3[:, :, 0, :], func=Ident,
                                     bias=bias)
                nc.scalar.activation(out=d1, in_=pr3[:, :, 1, :], func=Ident,
                                     bias=bias)
            else:
                nc.vector.tensor_scalar_add(out=d0, in0=pr3[:, :, 0, :],
                                            scalar1=bias)
                nc.vector.tensor_scalar_add(out=d1, in0=pr3[:, :, 1, :],
                                            scalar1=bias)

        triu_b = triu_sb.unsqueeze(1)

        with tc.tile_pool(name="sc", bufs=2, space="PSUM") as scp, \
             tc.tile_pool(name="po", bufs=2, space="PSUM") as pop, \
             tc.tile_pool(name="aps", bufs=2, space="PSUM") as aps, \
             tc.tile_pool(name="pt", bufs=12) as ptp, \
             tc.tile_pool(name="ost", bufs=3) as ost, \
             tc.tile_pool(name="nrm", bufs=6) as nrm:

            def a01_chunk(n):
                xn = [xn_sl(k, n) for k in range(6)]
                psa = aps.tile([128, 512], f32, tag="apsA")
                for k in range(6):
                    nc.tensor.matmul(psa, w1_sb[k][:, 0:128], xn[k],
                                     start=(k == 0), stop=(k == 5))
                permuted_copy(qAB, psa, n, btA_sb)
                psb = aps.tile([128, 512], f32, tag="apsA")
                for k in range(6):
                    nc.tensor.matmul(psb, w1_sb[k][:, 128:256], xn[k],
                                     start=(k == 0), stop=(k == 5))
                nc.vector.tensor_scalar_add(
                    out=kAB[:, 512 * n:512 * (n + 1)], in0=psb,
                    scalar1=btB_sb)

            def a2_chunk(j):
                """j 0..3: qC2 chunk j; j 4..7: kC2 chunk j-4."""
                n = j % 4
                xn = [xn_sl(k, n) for k in range(6)]
                ps = aps.tile([128, 512], f32, tag="apsA")
                cols = slice(0, 128) if j < 4 else slice(128, 256)
                for k in range(6):
                    nc.tensor.matmul(ps, w2_sb[k][:, cols], xn[k],
                                     start=(k == 0), stop=(k == 5))
                if j < 4:
                    permuted_copy(qC2, ps, n, btC_sb)
                else:
                    nc.vector.tensor_scalar_add(
                        out=kC2[:, 512 * n:512 * (n + 1)], in0=ps,
                        scalar1=btD_sb)

            def v_proj(st):
                psv = aps.tile([128, 192], f32, tag="apsA")
                for k in range(6):
                    nc.tensor.matmul(psv, xst_sl(k, st), wv_sb[k],
                                     start=(k == 0), stop=(k == 5))
                vt = v_sb[st]
                nc.vector.tensor_tensor(
                    out=vt.rearrange("p (h c) -> p h c", c=65)[:, :, 0:64],
                    in0=psv.rearrange("p (h c) -> p h c", c=64),
                    in1=bvb_sb.rearrange("p (h c) -> p h c", c=64),
                    op=mybir.AluOpType.add)

            def norm(po, dst_rows, dst_cols, den_eng=None):
                """po [65,512]: rows 0-63 = out^T, row 64 = denom."""
                den = nrm.tile([1, 512], f32r, tag="den")
                if den_eng == "act":
                    nc.scalar.copy(out=den, in_=po[64:65, :])
                else:
                    nc.vector.tensor_copy(out=den, in_=po[64:65, :])
                dps = aps.tile([64, 512], f32, tag="apsA")
                nc.tensor.matmul(dps, onesr_sb, den, start=True, stop=True)
                rec = nrm.tile([64, 512], f32, tag="rec")
                nc.vector.reciprocal_approx_fast(out=rec, in_=dps)
                nc.vector.tensor_tensor(
                    out=dst_rows[:, dst_cols], in0=po[0:64, :], in1=rec,
                    op=mult)

            def exp_unit(sc, mask_lo, mask_hi):
                pt = ptp.tile([128, 1024], bf16, tag="pt")
                nc.scalar.activation(out=pt, in_=sc, func=Exp, scale=0.125)
                if mask_lo and mask_hi:
                    p3 = pt.rearrange("p (c k) -> p c k", k=128)
                    nc.vector.tensor_mul(
                        out=p3, in0=p3,
                        in1=triu_b.broadcast_to([128, 8, 128]))
                elif mask_lo or mask_hi:
                    off = 0 if mask_lo else 512
                    p3 = pt[:, off:off + 512].rearrange(
                        "p (c k) -> p c k", k=128)
                    nc.vector.tensor_mul(
                        out=p3, in0=p3,
                        in1=triu_b.broadcast_to([128, 4, 128]))
                return pt

            # ---- B01: heads h0,h1 on partition strips, PV delayed 1 unit ----
            def b01(g, hh, pre=None, den_eng=None):
                qc = slice(1024 * g + 512 * hh, 1024 * g + 512 * hh + 512)
                po0 = pop.tile([65, 512], f32, tag="po")
                po1 = pop.tile([65, 512], f32, tag="po")
                units = []  # (kslice, v_idx, masked)
                for w in range(NW):
                    units.append((slice(WIN * w, WIN * w + 128), 2 * w,
                                  g == 0))
                    if g == 1:
                        units.append((slice(WIN * w + 128, WIN * w + 256),
                                      2 * w + 1, True))
                nu = len(units)

                def pv(i, pt):
                    vsl = v_sb[units[i][1]]
                    nc.tensor.matmul(po0, vsl[:, 0:65], pt[:, 0:512],
                                     start=(i == 0), stop=(i == nu - 1))
                    nc.tensor.matmul(po1, vsl[:, 65:130], pt[:, 512:1024],
                                     start=(i == 0), stop=(i == nu - 1))

                prev = None
                for i, (ksl, vidx, msk) in enumerate(units):
                    sc = scp.tile([128, 1024], f32, tag="sc")
                    nc.tensor.matmul(sc[:, 0:512], kAB[0:64, ksl],
                                     qAB[0:64, qc], start=True, stop=True)
                    nc.tensor.matmul(sc[:, 512:1024], kAB[64:128, ksl],
                                     qAB[64:128, qc], start=True, stop=True)
                    pt = exp_unit(sc, mask_lo=msk, mask_hi=msk)
                    if prev is not None:
                        pv(*prev)
                    if pre is not None:
                        pre(i)
                    prev = (i, pt)
                pv(*prev)
                cols = slice(1024 * g + 512 * hh, 1024 * g + 512 * hh + 512)
                norm(po0, attn01[0:64, :], cols, den_eng)
                norm(po1, attn01[64:128, :], cols, den_eng)

            # ---- B2: head h2 quarter-pass; two half-blocks per sc tile ----
            def b2q(g, hh, pre=None, den_eng=None):
                qc = slice(1024 * g + 512 * hh, 1024 * g + 512 * hh + 512)
                pg = pop.tile([65, 512], f32, tag="po")
                halves = []  # (kslice, v_idx, masked)
                for w in range(NW):
                    if g == 0:
                        halves.append((slice(WIN * w, WIN * w + 128), 2 * w,
                                       True))
                    else:
                        halves.append((slice(WIN * w, WIN * w + 128), 2 * w,
                                       False))
                        halves.append((slice(WIN * w + 128, WIN * w + 256),
                                       2 * w + 1, True))
                nh = len(halves)

                def pvs(j, pt):
                    for s in (0, 1):
                        i = j + s
                        nc.tensor.matmul(
                            pg, v_sb[halves[i][1]][:, 130:195],
                            pt[:, 512 * s:512 * (s + 1)],
                            start=(i == 0), stop=(i == nh - 1))

                prev = None
                for j in range(0, nh, 2):
                    pair = halves[j:j + 2]
                    sc = scp.tile([128, 1024], f32, tag="sc")
                    for s, (ksl, _, _) in enumerate(pair):
                        nc.tensor.matmul(
                            sc[:, 512 * s:512 * (s + 1)],
                            kC2[64 * s:64 * (s + 1), ksl],
                            qC2[64 * s:64 * (s + 1), qc],
                            start=True, stop=True)
                    pt = exp_unit(sc, mask_lo=pair[0][2], mask_hi=pair[1][2])
                    if prev is not None:
                        pvs(*prev)
                    if pre is not None:
                        pre(j // 2)
                    prev = (j, pt)
                pvs(*prev)
                norm(pg, attn2[0:64, :], qc, den_eng)
                nc.vector.tensor_copy(out=attn2[64:128, qc],
                                      in_=attn2[0:64, qc])

            # ---- C: O-projection for one permuted 128-col tile ----
            def c_tile(p):
                pso = scp.tile([128, D], f32, tag="sc")
                pcols = slice(128 * p, 128 * (p + 1))
                half = 64 * (p % 2)  # alternate attn2/wo2 strip per p parity
                for (n0, n1) in ((0, 512), (512, 768)):
                    nc.tensor.matmul(pso[:, n0:n1], attn01[:, pcols],
                                     wo1_sb[:, n0:n1], start=True, stop=False)
                    nc.tensor.matmul(pso[:, n0:n1],
                                     attn2[half:half + 64, pcols],
                                     wo2_sb[half:half + 64, n0:n1],
                                     start=False, stop=True)
                ot = ost.tile([128, D], bf16, tag="ot")
                if p % 2 == 0 or p >= 12:
                    nc.scalar.copy(out=ot, in_=pso)
                else:
                    nc.vector.tensor_copy(out=ot, in_=pso)
                t = 2 * p if p < 8 else 2 * (p - 8) + 1
                nc.sync.dma_start(out=out[128 * t:128 * (t + 1), :], in_=ot)

            # ---- emission schedule ----
            # PE warmup: dummy matmuls on a memset tile, no DMA dependency —
            # keeps the PE HAM activity monitor busy through the DMA lead-in
            # so stage A runs at 2.4 GHz instead of the cold 1.2 GHz.
            wu_in = ost.tile([128, 128], bf16, tag="wu")
            nc.vector.memset(wu_in, 0.0)
            wu_ps = scp.tile([128, 128], f32, tag="sc")
            for _ in range(60):
                nc.tensor.matmul(wu_ps, wu_in, wu_in, start=True, stop=True)
            for st in range(NT):
                nc.vector.tensor_copy(
                    out=v_sb[st].rearrange(
                        "p (h c) -> p h c", c=65)[:, :, 64:65],
                    in_=onesb_sb[:, 0:3].unsqueeze(2))

            a01_chunk(0)
            a01_chunk(1)

            def pre00(i):
                # even-indexed V tiles (the only ones grp0 needs) + rest of A01
                if i == 1:
                    a01_chunk(2)
                if i == 2:
                    a01_chunk(3)
                if i < NW:
                    v_proj(2 * i)

            def pre01(i):
                # odd V tiles + h2 projections, filler for the second g0 pass
                if i < NW:
                    v_proj(2 * i + 1)
                    a2_chunk(i)

            def pre10(i):
                if i % 2 == 0 and i // 2 < 8:
                    c_tile(i // 2)

            def pre21(i):
                if i < 4:
                    c_tile(8 + i)

            b01(0, 0, pre=pre00, den_eng="act")
            b01(0, 1, pre=pre01, den_eng="act")
            b2q(0, 0, den_eng="act")
            b2q(0, 1, den_eng="act")
            b01(1, 0, pre=pre10)
            b01(1, 1)
            b2q(1, 0)
            b2q(1, 1, pre=pre21)
            for p in range(12, 16):
                c_tile(p)

    nc.compile()
    return nc


def _prep_core_inputs(inputs, c):
    x = inputs["x"]
    Wq, bq = inputs["Wq"], inputs["bq"]
    Wk, bk = inputs["Wk"], inputs["bk"]
    Wv, bv = inputs["Wv"], inputs["bv"]
    Wo = inputs["Wo"]
    b = c // 4
    r0 = (c % 4) * DH  # first feature row of this core's 192-row head block

    xT = np.asarray(x[b]).T.astype(np.float32)  # [768, 2048]
    W1 = np.concatenate(
        [Wq[r0:r0 + 128].T, Wk[r0:r0 + 128].T], axis=1)
    q2 = Wq[r0 + 128:r0 + 192].T
    k2 = Wk[r0 + 128:r0 + 192].T
    W2 = np.concatenate([q2, q2, k2, k2], axis=1)
    Wvp = Wv[r0:r0 + 192].T
    # packed x: [128, (n 4, k 6, s 512)]
    xBp = np.zeros((128, 12288), np.float32)
    for n in range(4):
        for k in range(6):
            xBp[:, 3072 * n + 512 * k:3072 * n + 512 * (k + 1)] = \
                xT[128 * k:128 * (k + 1), 512 * n:512 * (n + 1)]
    # packed weights: wQ [128, (k 6, w1 256)]; wR [128, (k 6, [w2 256|wv 192])]
    wQp = np.zeros((128, 1536), np.float32)
    wRp = np.zeros((128, 2688), np.float32)
    for k in range(6):
        wQp[:, 256 * k:256 * (k + 1)] = W1[128 * k:128 * (k + 1)]
        wRp[:, 448 * k:448 * k + 256] = W2[128 * k:128 * (k + 1)]
        wRp[:, 448 * k + 256:448 * k + 448] = Wvp[128 * k:128 * (k + 1)]
    bq2 = bq[r0 + 128:r0 + 192]
    bk2 = bk[r0 + 128:r0 + 192]
    bvb = np.zeros((128, 192), np.float32)
    for h in range(NHC):
        bvb[:, 64 * h:64 * h + 64] = bv[r0 + 64 * h:r0 + 64 * (h + 1)][None, :]

    return dict(
        xB=xBp.astype(BF16), wQ=wQp.astype(BF16), wR=wRp.astype(BF16),
        wo1=np.ascontiguousarray(Wo[:, r0:r0 + 128].T).astype(BF16),
        wo2=np.ascontiguousarray(np.concatenate(
            [Wo[:, r0 + 128:r0 + 192].T] * 2, axis=0)).astype(BF16),
        btA=np.ascontiguousarray(bq[r0:r0 + 128].reshape(128, 1)).astype(np.float32),
        btB=np.ascontiguousarray(bk[r0:r0 + 128].reshape(128, 1)).astype(np.float32),
        btC=np.concatenate([bq2, bq2]).reshape(128, 1).astype(np.float32),
        btD=np.concatenate([bk2, bk2]).reshape(128, 1).astype(np.float32),
        bvb=np.ascontiguousarray(bvb),
        triu=np.triu(np.ones((128, 128), np.float32)).astype(BF16),
        onesb=np.ones((128, 64), np.float32).astype(BF16),
        onesr=np.ones((1, 64), np.float32),
    )


def _install_ntff_hook():
    """Register antenv.axon_hooks with a ctypes NTFF profile hook so
    run_bass_kernel_spmd(trace=True) can capture device-side exec time."""
    import types, ctypes, contextlib

    try:
        import antenv.axon_hooks  # noqa: F401
        return
    except ImportError:
        pass
    so_path = "/opt/axon/libaxon_pjrt.so"
    lib = ctypes.CDLL(so_path)
    if not hasattr(lib, "axon_start_nrt_profile"):
        return
    lib.axon_start_nrt_profile.argtypes = [
        ctypes.POINTER(ctypes.c_int64), ctypes.c_size_t]
    lib.axon_start_nrt_profile.restype = ctypes.c_int64
    lib.axon_stop_nrt_profile.argtypes = [ctypes.c_char_p]
    lib.axon_stop_nrt_profile.restype = ctypes.c_int64

    @contextlib.contextmanager
    def _hook(output_dir, device_ids):
        import jax
        jax.devices()
        if device_ids:
            ids = (ctypes.c_int64 * len(device_ids))(*device_ids)
            rc = lib.axon_start_nrt_profile(ids, len(device_ids))
        else:
            rc = lib.axon_start_nrt_profile(None, 0)
        if rc != 0:
            raise RuntimeError(f"axon_start_nrt_profile rc={rc}")
        try:
            yield
        finally:
            n = lib.axon_stop_nrt_profile(str(output_dir).encode())
            print(f"profile: {n} file(s) written to {output_dir}")

    mod = types.ModuleType("antenv.axon_hooks")
    mod.get_axon_ntff_profile_hook = lambda: _hook
    mod.set_axon_ntff_profile_hook = lambda h: None
    sys.modules["antenv.axon_hooks"] = mod
    import antenv
    antenv.axon_hooks = mod


def kernel(**inputs):
    import os
    from concourse import bass_utils

    if "nc" not in _CACHE:
        _CACHE["nc"] = _build_program()
    nc = _CACHE["nc"]

    trace = bool(os.environ.get("MHA_TRACE"))
    kwargs = {}
    if trace:
        _install_ntff_hook()
        kwargs = dict(trace=True, tmpdir="/tmp/mha_trace")
        os.makedirs("/tmp/mha_trace", exist_ok=True)

    in_maps = [_prep_core_inputs(inputs, c) for c in range(8)]
    res = bass_utils.run_bass_kernel_spmd(
        nc, in_maps, core_ids=list(range(8)), **kwargs)
    _CACHE["last_results"] = res
    if trace and res.exec_time_ns is not None:
        print(f"HW exec time: {res.exec_time_ns} ns")
    out = np.zeros((B, S, D), np.float32)
    for c in range(8):
        out[c // 4] += res.results[c]["out"].astype(np.float32)
    out += np.asarray(inputs["bo"], np.float32).reshape(1, 1, D)
    return out
